# revision 11
# baseline (speedup 1.0000x reference)
"""Trainium2 Bass kernel for CombinedSARAFilter (fp16 blocked linear scan).

Math: with D_t = I_t - I_{t-1} (I_{-1}=0), the module reduces to
    x_t = lam_r x_{t-1} + p D_t + q I_t
    o_t = lam_d o_{t-1} + a_d x_t + c3 |D_t|      (out = o, since TAU_RA == TAU_D)
Blocked linear scan, time chunks of L=125 on SBUF partitions. The 2-row carry
state (x, o) is concatenated with the 126 chunk-input rows into a single
128-partition moving operand (carry on partitions 0:2), so each 512-lane block
needs only 3 matmuls:
    D   = WD^T  @ M          (chunk differences; zero weights on carry rows)
    O   = WIS^T @ M + WA^T @ (c3*|D|)
All I/O is fp16 (host converts f32<->fp16), halving HBM traffic; PSUM
accumulates in f32. ACT does the abs (+1 output copy), DVE the other PSUM->SBUF
output copies and the tiny all-SBUF carry forwards (4x copy mode). Chunk 0 uses
K=125 shifted weights applied straight to X[0:125], so no zero-init is needed.
"""
import sys

sys.path.insert(0, "/opt/trn_rl_repo")

import numpy as np

# filter constants
DT = 0.1
TAU_RA, K3 = 30.0, 2.0
TAU_R, TAU_D, K1, K2 = 5.0, 30.0, 0.05, 3.0
A_R = DT / TAU_R
A_D = DT / TAU_D
LAM_R = 1.0 - A_R
LAM_D = 1.0 - A_D
P = A_R * K2 / DT
Q = A_R * K1
C3 = K3 / TAU_RA

B, T, N = 8, 2000, 2048
L = 125            # time chunk (on partitions)
NCH = T // L       # 16
NB = 512           # lane block (PSUM bank = 512 fp32)
NBLK = N // NB     # 4


def build_weights():
    """Host-side fp64 construction of the chunk filter matrices (fp16 out)."""
    i = np.arange(L)
    Mr = np.tril(LAM_R ** (i[:, None] - i[None, :]))
    Md = np.tril(LAM_D ** (i[:, None] - i[None, :]))
    Bp = np.zeros((L, L + 1))
    Bp[i, i + 1] = 1.0
    Bp[i, i] = -1.0
    U = P * Bp
    U[:, 1:] += Q * np.eye(L)
    F1 = A_D * Md @ Mr @ U                  # [125, 126] o response to ihat
    v1 = LAM_D ** (i + 1)                   # o response to o_in
    v2 = A_D * (Md @ (LAM_R ** (i + 1)))    # o response to x_in
    xrow_I = (Mr @ U)[L - 1]                # [126] x_out response to ihat

    # Combined stationary [K=128, M=127].
    # K rows: 0 = x_in, 1 = o_in, 2..127 = ihat_0..125 (ihat_0 = prev last I)
    # M cols: 0 = x_out, 1 = o_out, 2..126 = out rows 0..124
    WIS_T = np.zeros((128, 127))
    WIS_T[0, 0] = LAM_R ** L
    WIS_T[0, 1] = v2[L - 1]
    WIS_T[0, 2:] = v2
    WIS_T[1, 1] = v1[L - 1]
    WIS_T[1, 2:] = v1
    WIS_T[2:, 0] = xrow_I
    WIS_T[2:, 1] = F1[L - 1]
    WIS_T[2:, 2:] = F1.T

    WD_T = np.zeros((128, 125))             # rows 0:2 zero (carry ignored)
    WD_T[2:, :] = Bp.T

    W_A = np.zeros((127, L))                # abs-path response, lhsT = W_A.T
    W_A[1] = Md[L - 1]
    W_A[2:] = Md
    WA_T = np.ascontiguousarray(W_A.T)      # [125, 127]

    # Pack into one [128, 631] tensor: cols 0:125 = WD_T, 125:252 = WIS_T,
    # 252:379 = WA_T.  Chunk-0 variants (ihat_0 = 0, carry = 0 -> K=125
    # applied straight to X[0:125]): cols 379:504 = WD_T rows 3:128,
    # 504:631 = WIS_T rows 3:128.
    W = np.zeros((128, 631))
    W[0:128, 0:125] = WD_T
    W[0:128, 125:252] = WIS_T
    W[0:125, 252:379] = WA_T
    W[0:125, 379:504] = WD_T[3:128]
    W[0:125, 504:631] = WIS_T[3:128]
    return {"W": W.astype(np.float16)}


def build_program(reps: int = 1, mode: str = "full"):
    """Emit the single-core SPMD program. Returns (nc, weight_arrays)."""
    from concourse import bacc, mybir, tile

    dt = mybir.dt
    w = build_weights()

    nc = bacc.Bacc("TRN2", target_bir_lowering=False, debug=False)

    X = nc.dram_tensor("X", [T, N], dt.float16, kind="ExternalInput")
    Y = nc.dram_tensor("Y", [T, N], dt.float16, kind="ExternalOutput")
    wd = {
        name: nc.dram_tensor(name, list(arr.shape), dt.float16, kind="ExternalInput")
        for name, arr in w.items()
    }

    with tile.TileContext(nc) as tc:
        with (
            tc.tile_pool(name="wpool", bufs=1) as wpool,
            tc.tile_pool(name="mpool", bufs=4) as mpool,
            tc.tile_pool(name="opool", bufs=4) as opool,
            tc.tile_pool(name="apool", bufs=8) as apool,
            tc.tile_pool(name="psO", bufs=5, space="PSUM") as psO,
            tc.tile_pool(name="psD", bufs=3, space="PSUM") as psD,
        ):
            # weights -> SBUF once (single DMA)
            w_t = wpool.tile([128, 631], dt.float16, tag="W")
            nc.sync.dma_start(out=w_t[:], in_=wd["W"][:])
            wd_ap = w_t[0:128, 0:125]
            wis_ap = w_t[0:128, 125:252]
            wa_ap = w_t[0:125, 252:379]
            wd0_ap = w_t[0:125, 379:504]
            wis0_ap = w_t[0:125, 504:631]

            for rep in range(reps):
                m = [None] * NCH
                m[0] = mpool.tile([128, N], dt.float16, tag="m", name="m0")
                # chunk 0 uses the shifted K=125 weights on X[0:125] directly
                nc.scalar.dma_start(out=m[0][0:125, :], in_=X[0:L, :])

                for k in range(NCH):
                    if k + 1 < NCH:
                        m[k + 1] = mpool.tile(
                            [128, N], dt.float16, tag="m", name=f"m{k+1}"
                        )
                        nc.sync.dma_start(
                            out=m[k + 1][2:128, :],
                            in_=X[(k + 1) * L - 1:(k + 2) * L, :],
                        )

                    out_t = opool.tile([127, N], dt.float16, tag="out")
                    if mode == "dma":
                        for blk in range(NBLK):
                            c0 = blk * NB
                            nc.vector.tensor_copy(
                                out_t[0:125, c0:c0 + NB], m[k][2:127, c0:c0 + NB]
                            )
                        nc.scalar.dma_start(
                            out=Y[k * L:(k + 1) * L, :], in_=out_t[0:125, :]
                        )
                        continue

                    d_ps, a_tiles = [], []
                    for blk in range(NBLK):
                        c0 = blk * NB
                        dp = psD.tile([L, NB], dt.float32, tag="D")
                        if k == 0:
                            nc.tensor.matmul(
                                dp[:], wd0_ap, m[0][0:125, c0:c0 + NB],
                                start=True, stop=True,
                            )
                        else:
                            nc.tensor.matmul(
                                dp[:], wd_ap, m[k][:, c0:c0 + NB],
                                start=True, stop=True,
                            )
                        d_ps.append(dp)
                    for blk in range(NBLK):
                        a_ = apool.tile([L, NB], dt.float16, tag="A")
                        nc.scalar.activation(
                            a_[:], d_ps[blk][:],
                            func=mybir.ActivationFunctionType.Abs,
                            scale=float(C3),
                        )
                        a_tiles.append(a_)

                    o_ps = []
                    for blk in range(NBLK):
                        c0 = blk * NB
                        op = psO.tile([L + 2, NB], dt.float32, tag="O")
                        if k == 0:
                            nc.tensor.matmul(
                                op[:], wis0_ap, m[0][0:125, c0:c0 + NB],
                                start=True, stop=False,
                            )
                        else:
                            nc.tensor.matmul(
                                op[:], wis_ap, m[k][:, c0:c0 + NB],
                                start=True, stop=False,
                            )
                        o_ps.append(op)
                    for blk in range(NBLK):
                        nc.tensor.matmul(
                            o_ps[blk][:], wa_ap, a_tiles[blk][:],
                            start=False, stop=True,
                        )

                    last = k == NCH - 1
                    for blk in range(NBLK):
                        c0 = blk * NB
                        # PSUM -> fp16 staging: block 3 on ACT, rest on DVE
                        if blk == 3:
                            nc.scalar.copy(
                                out_t[:, c0:c0 + NB], o_ps[blk][:, :]
                            )
                        else:
                            nc.vector.tensor_copy(
                                out_t[:, c0:c0 + NB], o_ps[blk][:, :]
                            )
                        if not last:
                            # carry rows (all-SBUF fp16 -> DVE 4x copy mode)
                            nc.vector.tensor_copy(
                                m[k + 1][0:2, c0:c0 + NB],
                                out_t[0:2, c0:c0 + NB],
                            )
                        else:
                            # drain fast: per-block out-DMA, alternating queues
                            dma_eng = nc.sync if blk % 2 == 0 else nc.scalar
                            dma_eng.dma_start(
                                out=Y[k * L:(k + 1) * L, c0:c0 + NB],
                                in_=out_t[2:127, c0:c0 + NB],
                            )
                    if not last:
                        nc.sync.dma_start(
                            out=Y[k * L:(k + 1) * L, :], in_=out_t[2:127, :]
                        )

    nc.compile()
    return nc, w


_PROGRAM_CACHE = {}


def _get_program():
    if "nc" not in _PROGRAM_CACHE:
        nc, w = build_program()
        _PROGRAM_CACHE["nc"] = nc
        _PROGRAM_CACHE["w"] = w
    return _PROGRAM_CACHE["nc"], _PROGRAM_CACHE["w"]


def kernel(I_in: np.ndarray) -> np.ndarray:
    """Full-input entry point: I_in [8, 2000, 2048] fp32 -> out same shape."""
    from concourse.bass_utils import run_bass_kernel_spmd

    nc, w = _get_program()
    I16 = np.ascontiguousarray(I_in, dtype=np.float32).astype(np.float16)
    in_maps = [
        {"X": I16[b], **{name: arr for name, arr in w.items()}}
        for b in range(B)
    ]
    last_err = None
    for _attempt in range(3):
        try:
            res = run_bass_kernel_spmd(nc, in_maps, list(range(B)))
            return np.stack(
                [res.results[b]["Y"].astype(np.float32) for b in range(B)], axis=0
            )
        except Exception as e:  # transient device errors: retry
            last_err = e
            import time as _time
            _time.sleep(5)
    raise last_err


if __name__ == "__main__":
    rng = np.random.default_rng(0)
    I = rng.standard_normal((B, T, N), dtype=np.float32)
    out = kernel(I)
    print(out.shape, out.dtype, np.abs(out).max())


# revision 13
# speedup vs baseline: 1.2053x; 1.2053x over previous
"""Trainium2 Bass kernel for CombinedSARAFilter — polyphase-2 fp16 linear scan.

Same math as before (x/o recurrences -> chunked filter matmuls), but time is
packed TWO consecutive steps per SBUF partition so every HBM DMA descriptor
covers 8KB (two 4KB rows). This environment's DMA is descriptor-rate-bound,
so halving descriptor count nearly halves I/O time. Super-chunks of S=250
steps: moving operand M [128, 2*2048] (carry pair rows 0:2 duplicated per
parity half, 126 input pairs), four stationaries per path (parity-in x
parity-out). Outputs are staged pair-per-partition so out-DMA rows are 8KB
too. All I/O fp16; PSUM f32.
"""
import sys

sys.path.insert(0, "/opt/trn_rl_repo")

import numpy as np

DT = 0.1
TAU_RA, K3 = 30.0, 2.0
TAU_R, TAU_D, K1, K2 = 5.0, 30.0, 0.05, 3.0
A_R = DT / TAU_R
A_D = DT / TAU_D
LAM_R = 1.0 - A_R
LAM_D = 1.0 - A_D
P = A_R * K2 / DT
Q = A_R * K1
C3 = K3 / TAU_RA

B, T, N = 8, 2000, 2048
S = 250            # super-chunk time steps (125 pairs)
NSC = T // S       # 8
HP = 125           # output pairs per super-chunk
NB = 512
NBLK = N // NB     # 4
TP = T // 2        # 1000 pair-rows
N2 = 2 * N         # 4096


def build_poly_weights():
    i = np.arange(S)
    Mr2 = np.tril(LAM_R ** (i[:, None] - i[None, :]))
    Md2 = np.tril(LAM_D ** (i[:, None] - i[None, :]))
    Bp2 = np.zeros((S, S + 1)); Bp2[i, i + 1] = 1.0; Bp2[i, i] = -1.0
    U2 = P * Bp2; U2[:, 1:] += Q * np.eye(S)
    F1 = A_D * Md2 @ Mr2 @ U2            # [250, 251]
    v1 = LAM_D ** (i + 1)
    v2 = A_D * (Md2 @ (LAM_R ** (i + 1)))
    xrow = (Mr2 @ U2)[S - 1]

    mo = np.arange(HP)
    WIS = [[np.zeros((128, 127)) for _ in range(2)] for _ in range(2)]
    for po in range(2):
        tp = 2 * mo + po
        for q in range(2):
            Wq = WIS[po][q]
            if q == 0:
                Wq[0, 2:] = v2[tp]
                Wq[1, 2:] = v1[tp]
                if po == 1:
                    Wq[0, 0] = LAM_R ** S
                    Wq[0, 1] = v2[S - 1]
                    Wq[1, 1] = v1[S - 1]
            for p in range(126):
                jj = 2 * p + q - 1
                if 0 <= jj <= S:
                    Wq[2 + p, 2:] = F1[tp, jj]
                    if po == 1:
                        Wq[2 + p, 0] = xrow[jj]
                        Wq[2 + p, 1] = F1[S - 1, jj]

    WA = [[np.zeros((125, 127)) for _ in range(2)] for _ in range(2)]
    for po in range(2):
        tp = 2 * mo + po
        for dq in range(2):
            Wq = WA[po][dq]
            for p in range(HP):
                dd = 2 * p + dq
                Wq[p, 2:] = Md2[tp, dd]
                if po == 1:
                    Wq[p, 1] = Md2[S - 1, dd]

    WD = [[np.zeros((128, 125)) for _ in range(2)] for _ in range(2)]
    for dq in range(2):
        for pp in range(HP):
            WD[dq][dq][2 + pp + 1, pp] += 1.0
            WD[dq][1 - dq][2 + pp + dq, pp] -= 1.0

    WIS0 = [[np.zeros((125, 127)) for _ in range(2)] for _ in range(2)]
    for po in range(2):
        tp = 2 * mo + po
        for q in range(2):
            for p in range(125):
                jj = 2 * p + q + 1
                if jj <= S:
                    WIS0[po][q][p, 2:] = F1[tp, jj]
                    if po == 1:
                        WIS0[po][q][p, 0] = xrow[jj]
                        WIS0[po][q][p, 1] = F1[S - 1, jj]
    WD0 = [[np.zeros((125, 125)) for _ in range(2)] for _ in range(2)]
    for dq in range(2):
        for pp in range(HP):
            WD0[dq][dq][pp, pp] += 1.0
            if dq == 0:
                if pp >= 1:
                    WD0[0][1][pp - 1, pp] -= 1.0
            else:
                WD0[1][0][pp, pp] -= 1.0

    # pack all stationaries column-wise into one [128, total] tensor
    packed, offsets, col = [], {}, 0
    def pack(name, mats):
        nonlocal col
        for a_idx, row in enumerate(mats):
            for b_idx, mat in enumerate(row):
                kk, mm = mat.shape
                buf = np.zeros((128, mm))
                buf[0:kk] = mat
                packed.append(buf)
                offsets[(name, a_idx, b_idx)] = (col, mm, kk)
                col += mm
    pack("WIS", WIS)
    pack("WA", WA)
    pack("WD", WD)
    pack("WIS0", WIS0)
    pack("WD0", WD0)
    W = np.concatenate(packed, axis=1).astype(np.float16)
    return W, offsets


def build_program(reps: int = 1, mode: str = "full"):
    from concourse import bacc, mybir, tile

    dt = mybir.dt
    Warr, offs = build_poly_weights()

    nc = bacc.Bacc("TRN2", target_bir_lowering=False, debug=False)

    X = nc.dram_tensor("X", [TP, N2], dt.float16, kind="ExternalInput")
    Y = nc.dram_tensor("Y", [TP, N2], dt.float16, kind="ExternalOutput")
    Wd = nc.dram_tensor("W", list(Warr.shape), dt.float16, kind="ExternalInput")

    with tile.TileContext(nc) as tc:
        with (
            tc.tile_pool(name="wpool", bufs=1) as wpool,
            tc.tile_pool(name="mpool", bufs=3) as mpool,
            tc.tile_pool(name="opool", bufs=3) as opool,
            tc.tile_pool(name="apool", bufs=2) as apool,
            tc.tile_pool(name="psO", bufs=5, space="PSUM") as psO,
            tc.tile_pool(name="psD", bufs=3, space="PSUM") as psD,
        ):
            w_t = wpool.tile([128, Warr.shape[1]], dt.float16, tag="W")
            nc.sync.dma_start(out=w_t[:], in_=Wd[:])

            def wap(name, a_idx, b_idx):
                c0, mm, kk = offs[(name, a_idx, b_idx)]
                return w_t[0:kk, c0:c0 + mm]

            for rep in range(reps):
                m = [None] * NSC
                m[0] = mpool.tile([128, N2], dt.float16, tag="m", name="m0")
                nc.scalar.dma_start(out=m[0][0:125, :], in_=X[0:125, :])

                for g in range(NSC):
                    if g + 1 < NSC:
                        m[g + 1] = mpool.tile(
                            [128, N2], dt.float16, tag="m", name=f"m{g+1}"
                        )
                        nc.sync.dma_start(
                            out=m[g + 1][2:128, :],
                            in_=X[125 * (g + 1) - 1:125 * (g + 1) + 125, :],
                        )

                    out_t = opool.tile([127, N2], dt.float16, tag="out")
                    if mode == "dma":
                        for blk in range(NBLK):
                            c0 = blk * NB
                            nc.vector.tensor_copy(
                                out_t[0:125, c0:c0 + NB], m[g][0:125, c0:c0 + NB]
                            )
                            nc.vector.tensor_copy(
                                out_t[0:125, N + c0:N + c0 + NB],
                                m[g][0:125, N + c0:N + c0 + NB],
                            )
                        nc.scalar.dma_start(
                            out=Y[125 * g:125 * (g + 1), :], in_=out_t[0:125, :]
                        )
                        continue

                    K = 125 if g == 0 else 128
                    wis = "WIS0" if g == 0 else "WIS"
                    wdn = "WD0" if g == 0 else "WD"

                    # D blocks + abs -> a2 [125, 2*2048]
                    a2 = apool.tile([125, N2], dt.float16, tag="a2")
                    for blk in range(NBLK):
                        c0 = blk * NB
                        for dq in range(2):
                            dp = psD.tile([HP, NB], dt.float32, tag="D")
                            nc.tensor.matmul(
                                dp[:], wap(wdn, dq, 0), m[g][0:K, c0:c0 + NB],
                                start=True, stop=False,
                            )
                            nc.tensor.matmul(
                                dp[:], wap(wdn, dq, 1), m[g][0:K, N + c0:N + c0 + NB],
                                start=False, stop=True,
                            )
                            nc.scalar.activation(
                                a2[:, dq * N + c0:dq * N + c0 + NB], dp[:],
                                func=mybir.ActivationFunctionType.Abs,
                                scale=float(C3),
                            )

                    # O blocks: [127, 512] per (po, blk)
                    o_ps = {}
                    for blk in range(NBLK):
                        c0 = blk * NB
                        for po in range(2):
                            op = psO.tile([127, NB], dt.float32, tag="O")
                            nc.tensor.matmul(
                                op[:], wap(wis, po, 0), m[g][0:K, c0:c0 + NB],
                                start=True, stop=False,
                            )
                            nc.tensor.matmul(
                                op[:], wap(wis, po, 1), m[g][0:K, N + c0:N + c0 + NB],
                                start=False, stop=False,
                            )
                            nc.tensor.matmul(
                                op[:], wap("WA", po, 0), a2[:, c0:c0 + NB],
                                start=False, stop=False,
                            )
                            nc.tensor.matmul(
                                op[:], wap("WA", po, 1), a2[:, N + c0:N + c0 + NB],
                                start=False, stop=True,
                            )
                            o_ps[(po, blk)] = op

                    for blk in range(NBLK):
                        c0 = blk * NB
                        for po in range(2):
                            nc.vector.tensor_copy(
                                out_t[:, po * N + c0:po * N + c0 + NB],
                                o_ps[(po, blk)][:, :],
                            )
                        if g + 1 < NSC:
                            # carry rows from po=1 block, duplicated per half
                            for q in range(2):
                                nc.vector.tensor_copy(
                                    m[g + 1][0:2, q * N + c0:q * N + c0 + NB],
                                    out_t[0:2, N + c0:N + c0 + NB],
                                )
                    nc.scalar.dma_start(
                        out=Y[125 * g:125 * (g + 1), :], in_=out_t[2:127, :]
                    )

    nc.compile()
    return nc, {"W": Warr}


_PROGRAM_CACHE = {}


def _get_program():
    if "nc" not in _PROGRAM_CACHE:
        nc, w = build_program()
        _PROGRAM_CACHE["nc"] = nc
        _PROGRAM_CACHE["w"] = w
    return _PROGRAM_CACHE["nc"], _PROGRAM_CACHE["w"]


def kernel(I_in: np.ndarray) -> np.ndarray:
    """Full-input entry point: I_in [8, 2000, 2048] fp32 -> out same shape."""
    from concourse.bass_utils import run_bass_kernel_spmd

    nc, w = _get_program()
    I16 = np.ascontiguousarray(I_in, dtype=np.float32).astype(np.float16)
    in_maps = [
        {"X": I16[b].reshape(TP, N2), **w} for b in range(B)
    ]
    last_err = None
    for _attempt in range(3):
        try:
            res = run_bass_kernel_spmd(nc, in_maps, list(range(B)))
            return np.stack(
                [
                    res.results[b]["Y"].reshape(T, N).astype(np.float32)
                    for b in range(B)
                ],
                axis=0,
            )
        except Exception as e:  # transient device errors: retry
            last_err = e
            import time as _time
            _time.sleep(5)
    raise last_err


if __name__ == "__main__":
    rng = np.random.default_rng(0)
    I = rng.standard_normal((B, T, N), dtype=np.float32)
    out = kernel(I)
    print(out.shape, out.dtype, np.abs(out).max())


# revision 14
# speedup vs baseline: 2.9864x; 2.4777x over previous
"""Trainium2 Bass kernel for CombinedSARAFilter — polyphase-2 fp16 linear scan.

Same math as before (x/o recurrences -> chunked filter matmuls), but time is
packed TWO consecutive steps per SBUF partition so every HBM DMA descriptor
covers 8KB (two 4KB rows). This environment's DMA is descriptor-rate-bound,
so halving descriptor count nearly halves I/O time. Super-chunks of S=250
steps: moving operand M [128, 2*2048] (carry pair rows 0:2 duplicated per
parity half, 126 input pairs), four stationaries per path (parity-in x
parity-out). Outputs are staged pair-per-partition so out-DMA rows are 8KB
too. All I/O fp16; PSUM f32.
"""
import sys

sys.path.insert(0, "/opt/trn_rl_repo")

import numpy as np

DT = 0.1
TAU_RA, K3 = 30.0, 2.0
TAU_R, TAU_D, K1, K2 = 5.0, 30.0, 0.05, 3.0
A_R = DT / TAU_R
A_D = DT / TAU_D
LAM_R = 1.0 - A_R
LAM_D = 1.0 - A_D
P = A_R * K2 / DT
Q = A_R * K1
C3 = K3 / TAU_RA

B, T, N = 8, 2000, 2048
S = 250            # super-chunk time steps (125 pairs)
NSC = T // S       # 8
HP = 125           # output pairs per super-chunk
NB = 512
NBLK = N // NB     # 4
TP = T // 2        # 1000 pair-rows
N2 = 2 * N         # 4096


def build_poly_weights():
    i = np.arange(S)
    Mr2 = np.tril(LAM_R ** (i[:, None] - i[None, :]))
    Md2 = np.tril(LAM_D ** (i[:, None] - i[None, :]))
    Bp2 = np.zeros((S, S + 1)); Bp2[i, i + 1] = 1.0; Bp2[i, i] = -1.0
    U2 = P * Bp2; U2[:, 1:] += Q * np.eye(S)
    F1 = A_D * Md2 @ Mr2 @ U2            # [250, 251]
    v1 = LAM_D ** (i + 1)
    v2 = A_D * (Md2 @ (LAM_R ** (i + 1)))
    xrow = (Mr2 @ U2)[S - 1]

    mo = np.arange(HP)
    WIS = [[np.zeros((128, 127)) for _ in range(2)] for _ in range(2)]
    for po in range(2):
        tp = 2 * mo + po
        for q in range(2):
            Wq = WIS[po][q]
            if q == 0:
                Wq[0, 2:] = v2[tp]
                Wq[1, 2:] = v1[tp]
                if po == 1:
                    Wq[0, 0] = LAM_R ** S
                    Wq[0, 1] = v2[S - 1]
                    Wq[1, 1] = v1[S - 1]
            for p in range(126):
                jj = 2 * p + q - 1
                if 0 <= jj <= S:
                    Wq[2 + p, 2:] = F1[tp, jj]
                    if po == 1:
                        Wq[2 + p, 0] = xrow[jj]
                        Wq[2 + p, 1] = F1[S - 1, jj]

    WA = [[np.zeros((125, 127)) for _ in range(2)] for _ in range(2)]
    for po in range(2):
        tp = 2 * mo + po
        for dq in range(2):
            Wq = WA[po][dq]
            for p in range(HP):
                dd = 2 * p + dq
                Wq[p, 2:] = Md2[tp, dd]
                if po == 1:
                    Wq[p, 1] = Md2[S - 1, dd]

    WD = [[np.zeros((128, 125)) for _ in range(2)] for _ in range(2)]
    for dq in range(2):
        for pp in range(HP):
            WD[dq][dq][2 + pp + 1, pp] += 1.0
            WD[dq][1 - dq][2 + pp + dq, pp] -= 1.0

    WIS0 = [[np.zeros((125, 127)) for _ in range(2)] for _ in range(2)]
    for po in range(2):
        tp = 2 * mo + po
        for q in range(2):
            for p in range(125):
                jj = 2 * p + q + 1
                if jj <= S:
                    WIS0[po][q][p, 2:] = F1[tp, jj]
                    if po == 1:
                        WIS0[po][q][p, 0] = xrow[jj]
                        WIS0[po][q][p, 1] = F1[S - 1, jj]
    WD0 = [[np.zeros((125, 125)) for _ in range(2)] for _ in range(2)]
    for dq in range(2):
        for pp in range(HP):
            WD0[dq][dq][pp, pp] += 1.0
            if dq == 0:
                if pp >= 1:
                    WD0[0][1][pp - 1, pp] -= 1.0
            else:
                WD0[1][0][pp, pp] -= 1.0

    # pack all stationaries column-wise into one [128, total] tensor
    packed, offsets, col = [], {}, 0
    def pack(name, mats):
        nonlocal col
        for a_idx, row in enumerate(mats):
            for b_idx, mat in enumerate(row):
                kk, mm = mat.shape
                buf = np.zeros((128, mm))
                buf[0:kk] = mat
                packed.append(buf)
                offsets[(name, a_idx, b_idx)] = (col, mm, kk)
                col += mm
    pack("WIS", WIS)
    pack("WA", WA)
    pack("WD", WD)
    pack("WIS0", WIS0)
    pack("WD0", WD0)
    W = np.concatenate(packed, axis=1).astype(np.float16)
    return W, offsets


def build_program(reps: int = 1, mode: str = "full"):
    from concourse import bacc, mybir, tile

    dt = mybir.dt
    Warr, offs = build_poly_weights()

    nc = bacc.Bacc("TRN2", target_bir_lowering=False, debug=False)

    X = nc.dram_tensor("X", [TP, N2], dt.float16, kind="ExternalInput")
    Y = nc.dram_tensor("Y", [TP, N2], dt.float16, kind="ExternalOutput")
    Wd = nc.dram_tensor("W", list(Warr.shape), dt.float16, kind="ExternalInput")

    with tile.TileContext(nc) as tc:
        with (
            tc.tile_pool(name="wpool", bufs=1) as wpool,
            tc.tile_pool(name="mpool", bufs=4) as mpool,
            tc.tile_pool(name="opool", bufs=4) as opool,
            tc.tile_pool(name="apool", bufs=3) as apool,
            tc.tile_pool(name="psO", bufs=5, space="PSUM") as psO,
            tc.tile_pool(name="psD", bufs=3, space="PSUM") as psD,
        ):
            w_t = wpool.tile([128, Warr.shape[1]], dt.float16, tag="W")
            nc.sync.dma_start(out=w_t[:], in_=Wd[:])

            def wap(name, a_idx, b_idx):
                c0, mm, kk = offs[(name, a_idx, b_idx)]
                return w_t[0:kk, c0:c0 + mm]

            for rep in range(reps):
                m = [None] * NSC
                m[0] = mpool.tile([128, N2], dt.float16, tag="m", name="m0")
                nc.scalar.dma_start(out=m[0][0:125, :], in_=X[0:125, :])

                for g in range(NSC):
                    if g + 1 < NSC:
                        m[g + 1] = mpool.tile(
                            [128, N2], dt.float16, tag="m", name=f"m{g+1}"
                        )
                        nc.sync.dma_start(
                            out=m[g + 1][2:128, :],
                            in_=X[125 * (g + 1) - 1:125 * (g + 1) + 125, :],
                        )

                    out_t = opool.tile([127, N2], dt.float16, tag="out")
                    if mode == "dma":
                        for blk in range(NBLK):
                            c0 = blk * NB
                            nc.vector.tensor_copy(
                                out_t[0:125, c0:c0 + NB], m[g][0:125, c0:c0 + NB]
                            )
                            nc.vector.tensor_copy(
                                out_t[0:125, N + c0:N + c0 + NB],
                                m[g][0:125, N + c0:N + c0 + NB],
                            )
                        nc.scalar.dma_start(
                            out=Y[125 * g:125 * (g + 1), :], in_=out_t[0:125, :]
                        )
                        continue

                    K = 125 if g == 0 else 128
                    wis = "WIS0" if g == 0 else "WIS"
                    wdn = "WD0" if g == 0 else "WD"

                    # D blocks + abs -> a2 [125, 2*2048]
                    a2 = apool.tile([125, N2], dt.float16, tag="a2")
                    for blk in range(NBLK):
                        c0 = blk * NB
                        for dq in range(2):
                            dp = psD.tile([HP, NB], dt.float32, tag="D")
                            nc.tensor.matmul(
                                dp[:], wap(wdn, dq, 0), m[g][0:K, c0:c0 + NB],
                                start=True, stop=False,
                            )
                            nc.tensor.matmul(
                                dp[:], wap(wdn, dq, 1), m[g][0:K, N + c0:N + c0 + NB],
                                start=False, stop=True,
                            )
                            nc.scalar.activation(
                                a2[:, dq * N + c0:dq * N + c0 + NB], dp[:],
                                func=mybir.ActivationFunctionType.Abs,
                                scale=float(C3),
                            )

                    # O blocks: [127, 512] per (po, blk)
                    o_ps = {}
                    for blk in range(NBLK):
                        c0 = blk * NB
                        for po in range(2):
                            op = psO.tile([127, NB], dt.float32, tag="O")
                            nc.tensor.matmul(
                                op[:], wap(wis, po, 0), m[g][0:K, c0:c0 + NB],
                                start=True, stop=False,
                            )
                            nc.tensor.matmul(
                                op[:], wap(wis, po, 1), m[g][0:K, N + c0:N + c0 + NB],
                                start=False, stop=False,
                            )
                            nc.tensor.matmul(
                                op[:], wap("WA", po, 0), a2[:, c0:c0 + NB],
                                start=False, stop=False,
                            )
                            nc.tensor.matmul(
                                op[:], wap("WA", po, 1), a2[:, N + c0:N + c0 + NB],
                                start=False, stop=True,
                            )
                            o_ps[(po, blk)] = op

                    for blk in range(NBLK):
                        c0 = blk * NB
                        for po in range(2):
                            nc.vector.tensor_copy(
                                out_t[:, po * N + c0:po * N + c0 + NB],
                                o_ps[(po, blk)][:, :],
                            )
                        if g + 1 < NSC:
                            # carry rows from po=1 block, duplicated per half
                            for q in range(2):
                                nc.vector.tensor_copy(
                                    m[g + 1][0:2, q * N + c0:q * N + c0 + NB],
                                    out_t[0:2, N + c0:N + c0 + NB],
                                )
                    out_eng = nc.scalar if g % 2 == 0 else nc.sync
                    out_eng.dma_start(
                        out=Y[125 * g:125 * (g + 1), :], in_=out_t[2:127, :]
                    )

    nc.compile()
    return nc, {"W": Warr}


_PROGRAM_CACHE = {}


def _get_program():
    if "nc" not in _PROGRAM_CACHE:
        nc, w = build_program()
        _PROGRAM_CACHE["nc"] = nc
        _PROGRAM_CACHE["w"] = w
    return _PROGRAM_CACHE["nc"], _PROGRAM_CACHE["w"]


def kernel(I_in: np.ndarray) -> np.ndarray:
    """Full-input entry point: I_in [8, 2000, 2048] fp32 -> out same shape."""
    from concourse.bass_utils import run_bass_kernel_spmd

    nc, w = _get_program()
    I16 = np.ascontiguousarray(I_in, dtype=np.float32).astype(np.float16)
    in_maps = [
        {"X": I16[b].reshape(TP, N2), **w} for b in range(B)
    ]
    last_err = None
    for _attempt in range(3):
        try:
            res = run_bass_kernel_spmd(nc, in_maps, list(range(B)))
            return np.stack(
                [
                    res.results[b]["Y"].reshape(T, N).astype(np.float32)
                    for b in range(B)
                ],
                axis=0,
            )
        except Exception as e:  # transient device errors: retry
            last_err = e
            import time as _time
            _time.sleep(5)
    raise last_err


if __name__ == "__main__":
    rng = np.random.default_rng(0)
    I = rng.standard_normal((B, T, N), dtype=np.float32)
    out = kernel(I)
    print(out.shape, out.dtype, np.abs(out).max())
